# revision 1
# baseline (speedup 1.0000x reference)
"""Adaptive avg pool 2D (16,768,64,48) -> (16,768,7,7) on 8 TRN2 NeuronCores.

Data-parallel over B*C rows: 12288 rows of 64*48=3072 f32, 1536 rows/core.
Per 128-row tile: H-pool (one strided reduce_sum; windows all size 10,
stride 9) then W-pool (two grouped reduce_sums; sizes 7/8), then a
per-element scale. Stores ride gpsimd so their DVE waits never stall the
SP load pipeline; a post-Tile pass legalizes multi-wait sync for this
walrus (max 1 wait/instruction, 2 on EventSemaphore).
  W windows (48->7): q=0:[0,7) q=6:[41,48) size 7; q=1..5 start 6+7(q-1) size 8
  H windows (64->7): start 9*o, size 10 for all o
"""

import sys

_TRN_REPO = "/opt/trn_rl_repo"
if _TRN_REPO not in sys.path:
    sys.path.insert(0, _TRN_REPO)

import numpy as np

import concourse.bass as bass
import concourse.mybir as mybir
from concourse.tile import TileContext

B, C, H, W = 16, 768, 64, 48
HO, WO = 7, 7
NCORES = 8
ROWS = B * C // NCORES  # 1536 rows per core
P = 128
NTILES = ROWS // P  # 12
SPLIT_LAST = 2  # how many trailing tiles use the two-chunk load

_nc_cache = None


def _legalize_multiwait(nc: bass.Bass) -> None:
    """Walrus (this version) accepts at most one sync wait per instruction
    (two for EventSemaphore). Tile's sem assignment can emit more (e.g. the
    kernel-tail drain waits on every DMA queue sem). Hoist all but the last
    wait into dedicated single-wait EventSemaphore carriers placed directly
    before the offending instruction on the same engine."""
    n = 0
    for b in nc.m.functions[0].blocks:
        insts = b.instructions
        i = 0
        while i < len(insts):
            inst = insts[i]
            si = inst.sync_info
            if si is not None and len(si.on_wait) > 1:
                waits = list(si.on_wait)
                carriers = []
                rest = waits[:-1]
                # EventSemaphore carriers can hold 2 waits each.
                for j in range(0, len(rest), 2):
                    n += 1
                    ev = mybir.InstEventSemaphore(
                        name=f"I-waitfix-{n}", ins=[], outs=[]
                    )
                    ev.engine = inst.engine
                    ev.sync_info = mybir.SyncInfo(
                        on_wait=rest[j : j + 2], on_update=[]
                    )
                    nc.register_instruction(ev)
                    carriers.append(ev)
                inst.sync_info = mybir.SyncInfo(
                    on_wait=[waits[-1]], on_update=list(si.on_update)
                )
                insts[i:i] = carriers
                i += len(carriers)
            i += 1


def _build() -> bass.Bass:
    nc = bass.Bass()
    x = nc.dram_tensor("x", [ROWS, H * W], mybir.dt.float32, kind="ExternalInput")
    out = nc.dram_tensor(
        "out", [ROWS, HO * WO], mybir.dt.float32, kind="ExternalOutput"
    )
    f32 = mybir.dt.float32
    X = mybir.AxisListType.X
    with TileContext(nc) as tc:
        with (
            tc.tile_pool(name="xp", bufs=NTILES) as xp,
            tc.tile_pool(name="tp", bufs=3) as tp,
            tc.tile_pool(name="op", bufs=3) as op,
            tc.tile_pool(name="sp", bufs=NTILES) as sp,
            tc.tile_pool(name="cp", bufs=1) as cp,
        ):
            # Scale tile: sc[p, o*7+q] = 1/(10 * wsize_q); wsize = 7 for
            # q in {0,6}, 8 for q in 1..5. Same for every o.
            sc = cp.tile([P, HO * WO], f32)
            ps = list(sc.ap[0])
            nc.vector.memset(
                bass.AP(tensor=sc.tensor, offset=sc.offset, ap=[ps, [WO, HO], [6, 2]]),
                1.0 / 70.0,
            )
            nc.vector.memset(
                bass.AP(
                    tensor=sc.tensor, offset=sc.offset + 1, ap=[ps, [WO, HO], [1, 5]]
                ),
                1.0 / 80.0,
            )
            # Tiles >= NTILES - SPLIT_LAST load in two column chunks at the
            # h=27 window boundary (o 0-2 need h<28, o 3-6 need h>=27), so
            # the H-reduce of chunk A overlaps chunk B's transfer. This lets
            # DVE converge from load+5.1us to load+3.7us over the last few
            # tiles, shrinking the post-stream tail.
            HA = 28  # rows 0..27 cover o=0..2
            HB = 27  # rows 27..63 cover o=3..6
            for i in range(NTILES):
                rows = x[i * P : (i + 1) * P, :].rearrange(
                    "p (h w) -> p h w", w=W
                )
                tH = tp.tile([P, HO, W], f32)
                ph = list(tH.ap[0])
                if i >= NTILES - SPLIT_LAST:
                    xa = xp.tile([P, HA, W], f32, tag="xa", bufs=2)
                    xb = xp.tile([P, H - HB, W], f32, tag="xb", bufs=2)
                    nc.sync.dma_start(out=xa, in_=rows[:, :HA, :])
                    nc.sync.dma_start(out=xb, in_=rows[:, HB:, :])
                    nc.vector.reduce_sum(
                        out=tH[:, 0:3, :],
                        in_=bass.AP(
                            tensor=xa.tensor,
                            offset=xa.offset,
                            ap=[list(xa.ap[0]), [9 * W, 3], [1, W], [W, 10]],
                        ),
                        axis=X,
                    )
                    nc.vector.reduce_sum(
                        out=tH[:, 3:7, :],
                        in_=bass.AP(
                            tensor=xb.tensor,
                            offset=xb.offset,
                            ap=[list(xb.ap[0]), [9 * W, 4], [1, W], [W, 10]],
                        ),
                        axis=X,
                    )
                else:
                    xt = xp.tile([P, H, W], f32)
                    nc.sync.dma_start(out=xt, in_=rows)
                    pt = list(xt.ap[0])
                    # H pool (all windows size 10, stride 9) in one reduce:
                    # tH[p, o, w] = sum_{h in [9o, 9o+10)} x[p, h, w]
                    nc.vector.reduce_sum(
                        out=tH,
                        in_=bass.AP(
                            tensor=xt.tensor,
                            offset=xt.offset,
                            ap=[pt, [9 * W, HO], [1, W], [W, 10]],
                        ),
                        axis=X,
                    )
                # W pool on tH: q in {0, 6} (size-7 windows at w = 0 and 41)
                ot = op.tile([P, HO, WO], f32)
                po = list(ot.ap[0])
                nc.vector.reduce_sum(
                    out=bass.AP(
                        tensor=ot.tensor,
                        offset=ot.offset,
                        ap=[po, [WO, HO], [6, 2]],
                    ),
                    in_=bass.AP(
                        tensor=tH.tensor,
                        offset=tH.offset,
                        ap=[ph, [W, HO], [41, 2], [1, 7]],
                    ),
                    axis=X,
                )
                # q in 1..5: size-8 windows starting at 6 + 7*(q-1)
                nc.vector.reduce_sum(
                    out=bass.AP(
                        tensor=ot.tensor,
                        offset=ot.offset + 1,
                        ap=[po, [WO, HO], [1, 5]],
                    ),
                    in_=bass.AP(
                        tensor=tH.tensor,
                        offset=tH.offset + 6,
                        ap=[ph, [W, HO], [7, 5], [1, 8]],
                    ),
                    axis=X,
                )
                os_ = sp.tile([P, HO * WO], f32)
                last = i == NTILES - 1
                if last:
                    # Tail path: DVE and SP are both idle by now. DVE mul
                    # avoids the cross-engine hop; SP store descgen (~0.6us)
                    # beats Pool SWDGE (~1.0us), and its DVE wait can no
                    # longer block loads (all issued).
                    nc.vector.tensor_mul(
                        os_, ot.rearrange("p a b -> p (a b)"), sc
                    )
                    nc.sync.dma_start(
                        out=out[i * P : (i + 1) * P, :],
                        in_=os_,
                    )
                else:
                    # Steady state: scale on gpsimd keeps DVE under the DMA
                    # period; store on gpsimd so its DVE wait never blocks
                    # SP load issue.
                    nc.gpsimd.tensor_mul(
                        os_, ot.rearrange("p a b -> p (a b)"), sc
                    )
                    nc.gpsimd.dma_start(
                        out=out[i * P : (i + 1) * P, :],
                        in_=os_,
                    )
    _legalize_multiwait(nc)
    return nc


def kernel(x: np.ndarray) -> np.ndarray:
    global _nc_cache
    from concourse.bass_utils import run_bass_kernel_spmd

    xr = np.ascontiguousarray(np.asarray(x, dtype=np.float32).reshape(B * C, H * W))
    if _nc_cache is None:
        _nc_cache = _build()
    nc = _nc_cache
    in_maps = [
        {"x": xr[k * ROWS : (k + 1) * ROWS]} for k in range(NCORES)
    ]
    res = run_bass_kernel_spmd(nc, in_maps, list(range(NCORES)))
    out = np.concatenate([r["out"] for r in res.results], axis=0)
    return out.reshape(B, C, HO, WO)



# revision 3
# speedup vs baseline: 1.0121x; 1.0121x over previous
"""Adaptive avg pool 2D (16,768,64,48) -> (16,768,7,7) on 8 TRN2 NeuronCores.

12 tiles of 128 rows per core. DMA device is the bottleneck (52.4us of
loads); plan: gap-free load stream, minimal post-load tail.

Steady tiles 0..10: DVE windowed H-reduce cols [0,38), Pool 9-add chain
cols [38,48), DVE W-pool (2 reduces), Act scales (1/70, 1/80) into one
[128, 588] output staging tile. Tile 11 loads as 3 h-chunks (o0-2 /
o3-5 / o6-last); chunk compute is fused 2D reduces on DVE; the final o6
chunk's scale runs on DVE and its 7 columns store from SP.

Stores are batched into 3 Act DMAs + 1 SP DMA, emitted late enough that
every store's DMA request reaches the device FIFO after the last load
request (the FIFO is grant-ordered; an early store would push loads
back). HWDGE-lane issue order: loads first, stores after.
"""

import sys

_TRN_REPO = "/opt/trn_rl_repo"
if _TRN_REPO not in sys.path:
    sys.path.insert(0, _TRN_REPO)

import numpy as np

import concourse.bass as bass
import concourse.mybir as mybir
from concourse.tile import TileContext

B, C, H, W = 16, 768, 64, 48
HO, WO = 7, 7
NCORES = 8
ROWS = B * C // NCORES  # 1536
P = 128
NTILES = ROWS // P  # 12
WD = 39  # steady: DVE H cols [0, WD), Pool cols [WD, 48)
f32 = mybir.dt.float32
X = mybir.AxisListType.X
XY = mybir.AxisListType.XY

_nc_cache = None


def _legalize_multiwait(nc: bass.Bass) -> None:
    """Walrus accepts at most one sync wait per instruction (two for
    EventSemaphore). Hoist extra waits into single-wait EventSemaphore
    carriers placed directly before the offending instruction."""
    n = 0
    for b in nc.m.functions[0].blocks:
        insts = b.instructions
        i = 0
        while i < len(insts):
            inst = insts[i]
            si = inst.sync_info
            if si is not None and len(si.on_wait) > 1:
                waits = sorted(
                    si.on_wait,
                    key=lambda w: 1 if (w.ant_name or "").startswith("DMA") else 0,
                )
                carriers = []
                rest = waits[:-1]
                for j in range(0, len(rest), 2):
                    n += 1
                    ev = mybir.InstEventSemaphore(
                        name=f"I-waitfix-{n}", ins=[], outs=[]
                    )
                    ev.engine = inst.engine
                    ev.sync_info = mybir.SyncInfo(
                        on_wait=rest[j : j + 2], on_update=[]
                    )
                    nc.register_instruction(ev)
                    carriers.append(ev)
                inst.sync_info = mybir.SyncInfo(
                    on_wait=[waits[-1]], on_update=list(si.on_update)
                )
                insts[i:i] = carriers
                i += len(carriers)
            i += 1



def _strip_pool_ring_memsets(nc: bass.Bass) -> None:
    """The framework preamble memsets the SWDGE descriptor rings on Pool.
    This kernel issues no SWDGE DMAs (loads ride SP, stores Act/SP via
    HWDGE), so the ring init only delays the entry barrier; drop it."""
    for b in nc.m.functions[0].blocks:
        keep = [
            i
            for i in b.instructions
            if not (
                type(i).__name__ == "InstMemset"
                and str(i.engine).endswith("Pool")
                and i.name in ("I-29", "I-30", "I-31", "I-32")
            )
        ]
        if len(keep) != len(b.instructions):
            b.instructions[:] = keep


def _build() -> bass.Bass:
    nc = bass.Bass()
    x = nc.dram_tensor("x", [ROWS, H * W], f32, kind="ExternalInput")
    out = nc.dram_tensor("out", [ROWS, HO * WO], f32, kind="ExternalOutput")
    ACopy = mybir.ActivationFunctionType.Copy
    with TileContext(nc) as tc:
        with (
            tc.tile_pool(name="xp", bufs=NTILES - 1) as xp,
            tc.tile_pool(name="cp", bufs=1) as cpool,
            tc.tile_pool(name="tp", bufs=3) as tp,
            tc.tile_pool(name="wp", bufs=3) as wp,
            tc.tile_pool(name="op", bufs=1) as op,
        ):
            nfull = NTILES - 1
            os_ = op.tile([P, NTILES * HO * WO], f32)  # [128, 588] staging
            po = list(os_.ap[0])

            # --- loads: 11 full tiles, then tile 11 in 3 chunks ----------
            xt = []
            for i in range(nfull):
                t = xp.tile([P, H, W], f32)
                nc.sync.dma_start(
                    out=t,
                    in_=x[i * P : (i + 1) * P, :].rearrange(
                        "p (h w) -> p h w", w=W
                    ),
                )
                xt.append(t)
            rows11 = x[nfull * P :, :].rearrange("p (h w) -> p h w", w=W)
            xa = cpool.tile([P, 28, W], f32, tag="xa")
            nc.sync.dma_start(out=xa, in_=rows11[:, 0:28, :])
            xb = cpool.tile([P, 28, W], f32, tag="xb")
            nc.sync.dma_start(out=xb, in_=rows11[:, 27:55, :])
            xc = cpool.tile([P, 10, W], f32, tag="xc")
            nc.sync.dma_start(out=xc, in_=rows11[:, 54:64, :])

            # --- helpers -------------------------------------------------
            def h_pool(xtile, tH):
                pt = list(xtile.ap[0])
                ph = list(tH.ap[0])
                nc.vector.reduce_sum(
                    out=bass.AP(
                        tensor=tH.tensor,
                        offset=tH.offset,
                        ap=[ph, [W, HO], [1, WD]],
                    ),
                    in_=bass.AP(
                        tensor=xtile.tensor,
                        offset=xtile.offset,
                        ap=[pt, [9 * W, HO], [1, WD], [W, 10]],
                    ),
                    axis=X,
                )
                tsl = bass.AP(
                    tensor=tH.tensor,
                    offset=tH.offset + WD,
                    ap=[ph, [W, HO], [1, W - WD]],
                )

                def xsl(k):
                    return bass.AP(
                        tensor=xtile.tensor,
                        offset=xtile.offset + k * W + WD,
                        ap=[pt, [9 * W, HO], [1, W - WD]],
                    )

                nc.gpsimd.tensor_add(tsl, xsl(0), xsl(1))
                for k in range(2, 10):
                    nc.gpsimd.tensor_add(tsl, tsl, xsl(k))

            def w_pool(tH, wout):
                ph = list(tH.ap[0])
                pw = list(wout.ap[0])
                nc.vector.reduce_sum(
                    out=bass.AP(
                        tensor=wout.tensor,
                        offset=wout.offset,
                        ap=[pw, [WO, HO], [6, 2]],
                    ),
                    in_=bass.AP(
                        tensor=tH.tensor,
                        offset=tH.offset,
                        ap=[ph, [W, HO], [41, 2], [1, 7]],
                    ),
                    axis=X,
                )
                nc.vector.reduce_sum(
                    out=bass.AP(
                        tensor=wout.tensor,
                        offset=wout.offset + 1,
                        ap=[pw, [WO, HO], [1, 5]],
                    ),
                    in_=bass.AP(
                        tensor=tH.tensor,
                        offset=tH.offset + 6,
                        ap=[ph, [W, HO], [7, 5], [1, 8]],
                    ),
                    axis=X,
                )

            def act_scales(wout, obase, o0, no):
                pw = list(wout.ap[0])
                nc.scalar.activation(
                    out=bass.AP(
                        tensor=os_.tensor,
                        offset=os_.offset + obase + 7 * o0,
                        ap=[po, [WO, no], [6, 2]],
                    ),
                    in_=bass.AP(
                        tensor=wout.tensor,
                        offset=wout.offset + 7 * o0,
                        ap=[pw, [WO, no], [6, 2]],
                    ),
                    func=ACopy,
                    scale=1.0 / 70.0,
                )
                nc.scalar.activation(
                    out=bass.AP(
                        tensor=os_.tensor,
                        offset=os_.offset + obase + 7 * o0 + 1,
                        ap=[po, [WO, no], [1, 5]],
                    ),
                    in_=bass.AP(
                        tensor=wout.tensor,
                        offset=wout.offset + 7 * o0 + 1,
                        ap=[pw, [WO, no], [1, 5]],
                    ),
                    func=ACopy,
                    scale=1.0 / 80.0,
                )

            def chunk_hw(xtile, tH, wout, o0, no, h0):
                """H-pool rows [o0,o0+no) split DVE/Pool, then W-pool."""
                pt = list(xtile.ap[0])
                ph = list(tH.ap[0])
                pw = list(wout.ap[0])
                off = (9 * o0 - h0) * W
                nc.vector.reduce_sum(
                    out=bass.AP(
                        tensor=tH.tensor,
                        offset=tH.offset + o0 * W,
                        ap=[ph, [W, no], [1, WD]],
                    ),
                    in_=bass.AP(
                        tensor=xtile.tensor,
                        offset=xtile.offset + off,
                        ap=[pt, [9 * W, no], [1, WD], [W, 10]],
                    ),
                    axis=X,
                )
                tsl = bass.AP(
                    tensor=tH.tensor,
                    offset=tH.offset + o0 * W + WD,
                    ap=[ph, [W, no], [1, W - WD]],
                )

                def xsl(k):
                    return bass.AP(
                        tensor=xtile.tensor,
                        offset=xtile.offset + off + k * W + WD,
                        ap=[pt, [9 * W, no], [1, W - WD]],
                    )

                nc.gpsimd.tensor_add(tsl, xsl(0), xsl(1))
                for k in range(2, 10):
                    nc.gpsimd.tensor_add(tsl, tsl, xsl(k))
                nc.vector.reduce_sum(
                    out=bass.AP(
                        tensor=wout.tensor,
                        offset=wout.offset + 7 * o0,
                        ap=[pw, [WO, no], [6, 2]],
                    ),
                    in_=bass.AP(
                        tensor=tH.tensor,
                        offset=tH.offset + o0 * W,
                        ap=[ph, [W, no], [41, 2], [1, 7]],
                    ),
                    axis=X,
                )
                nc.vector.reduce_sum(
                    out=bass.AP(
                        tensor=wout.tensor,
                        offset=wout.offset + 7 * o0 + 1,
                        ap=[pw, [WO, no], [1, 5]],
                    ),
                    in_=bass.AP(
                        tensor=tH.tensor,
                        offset=tH.offset + o0 * W + 6,
                        ap=[ph, [W, no], [7, 5], [1, 8]],
                    ),
                    axis=X,
                )

            def fused_chunk(xtile, wout, o0, no, h0):
                """wout[:, 7o+q] for o in [o0,o0+no) by 2D (h,w) reduces."""
                pc = list(xtile.ap[0])
                pw = list(wout.ap[0])
                off = (9 * o0 - h0) * W
                nc.vector.reduce_sum(
                    out=bass.AP(
                        tensor=wout.tensor,
                        offset=wout.offset + 7 * o0,
                        ap=[pw, [WO, no], [6, 2]],
                    ),
                    in_=bass.AP(
                        tensor=xtile.tensor,
                        offset=xtile.offset + off,
                        ap=[pc, [9 * W, no], [41, 2], [W, 10], [1, 7]],
                    ),
                    axis=XY,
                )
                nc.vector.reduce_sum(
                    out=bass.AP(
                        tensor=wout.tensor,
                        offset=wout.offset + 7 * o0 + 1,
                        ap=[pw, [WO, no], [1, 5]],
                    ),
                    in_=bass.AP(
                        tensor=xtile.tensor,
                        offset=xtile.offset + off + 6,
                        ap=[pc, [9 * W, no], [7, 5], [W, 10], [1, 8]],
                    ),
                    axis=XY,
                )

            # --- steady tiles 0..10 -------------------------------------
            for i in range(nfull):
                tH = tp.tile([P, HO, W], f32)
                wout = wp.tile([P, HO * WO], f32)
                h_pool(xt[i], tH)
                w_pool(tH, wout)
                act_scales(wout, i * HO * WO, 0, HO)
                if i == 8:
                    # Batched store, tiles 0..7. Emitted here so its DMA
                    # request trails every load request; its data (scales
                    # 0..7) is already ordered on Act.
                    nc.scalar.dma_start(
                        out=bass.AP(
                            tensor=out,
                            offset=0,
                            ap=[[HO * WO, P], [HO * WO * P, 8], [1, HO * WO]],
                        ),
                        in_=bass.AP(
                            tensor=os_.tensor,
                            offset=os_.offset,
                            ap=[po, [HO * WO, 8], [1, HO * WO]],
                        ),
                    )

            # --- tile 11 chunks -----------------------------------------
            wout11 = wp.tile([P, HO * WO], f32)
            tH11 = tp.tile([P, HO, W], f32)
            obase11 = nfull * HO * WO
            chunk_hw(xa, tH11, wout11, 0, 3, 0)
            act_scales(wout11, obase11, 0, 3)
            chunk_hw(xb, tH11, wout11, 3, 3, 27)
            act_scales(wout11, obase11, 3, 3)
            # tiles 8..10 full-block store once tile 10's scales are done
            nc.scalar.dma_start(
                out=bass.AP(
                    tensor=out,
                    offset=8 * HO * WO * P,
                    ap=[[HO * WO, P], [HO * WO * P, 3], [1, HO * WO]],
                ),
                in_=bass.AP(
                    tensor=os_.tensor,
                    offset=os_.offset + 8 * HO * WO,
                    ap=[po, [HO * WO, 3], [1, HO * WO]],
                ),
            )
            # o6: fused reduce + DVE scales + SP store (the tail chain)
            pc = list(xc.ap[0])
            pw = list(wout11.ap[0])
            nc.vector.reduce_sum(
                out=bass.AP(
                    tensor=wout11.tensor,
                    offset=wout11.offset + 42,
                    ap=[pw, [6, 2]],
                ),
                in_=bass.AP(
                    tensor=xc.tensor,
                    offset=xc.offset,
                    ap=[pc, [41, 2], [W, 10], [1, 7]],
                ),
                axis=XY,
            )
            nc.vector.reduce_sum(
                out=bass.AP(
                    tensor=wout11.tensor,
                    offset=wout11.offset + 43,
                    ap=[pw, [1, 5]],
                ),
                in_=bass.AP(
                    tensor=xc.tensor,
                    offset=xc.offset + 6,
                    ap=[pc, [7, 5], [W, 10], [1, 8]],
                ),
                axis=XY,
            )
            nc.vector.tensor_scalar_mul(
                bass.AP(
                    tensor=os_.tensor,
                    offset=os_.offset + obase11 + 42,
                    ap=[po, [6, 2]],
                ),
                bass.AP(
                    tensor=wout11.tensor,
                    offset=wout11.offset + 42,
                    ap=[pw, [6, 2]],
                ),
                1.0 / 70.0,
            )
            nc.vector.tensor_scalar_mul(
                bass.AP(
                    tensor=os_.tensor,
                    offset=os_.offset + obase11 + 43,
                    ap=[po, [1, 5]],
                ),
                bass.AP(
                    tensor=wout11.tensor,
                    offset=wout11.offset + 43,
                    ap=[pw, [1, 5]],
                ),
                1.0 / 80.0,
            )
            nc.sync.dma_start(
                out=out[nfull * P :, :],
                in_=os_[:, obase11 : obase11 + HO * WO],
            )
    _strip_pool_ring_memsets(nc)
    _legalize_multiwait(nc)
    return nc


def kernel(x: np.ndarray) -> np.ndarray:
    global _nc_cache
    from concourse.bass_utils import run_bass_kernel_spmd

    xr = np.ascontiguousarray(
        np.asarray(x, dtype=np.float32).reshape(B * C, H * W)
    )
    if _nc_cache is None:
        _nc_cache = _build()
    nc = _nc_cache
    in_maps = [{"x": xr[k * ROWS : (k + 1) * ROWS]} for k in range(NCORES)]
    res = run_bass_kernel_spmd(nc, in_maps, list(range(NCORES)))
    out = np.concatenate([r["out"] for r in res.results], axis=0)
    return out.reshape(B, C, HO, WO)


# revision 4
# speedup vs baseline: 1.0313x; 1.0189x over previous
"""Adaptive avg pool 2D (16,768,64,48) -> (16,768,7,7) on 8 TRN2 NeuronCores.

12 tiles of 128 rows per core. DMA device is the bottleneck (52.4us of
loads); plan: gap-free load stream, minimal post-load tail.

Steady tiles 0..10: DVE windowed H-reduce cols [0,38), Pool 9-add chain
cols [38,48), DVE W-pool (2 reduces), Act scales (1/70, 1/80) into one
[128, 588] output staging tile. Tile 11 loads as 3 h-chunks (o0-2 /
o3-5 / o6-last); chunk compute is fused 2D reduces on DVE; the final o6
chunk's scale runs on DVE and its 7 columns store from SP.

Stores are batched into 3 Act DMAs + 1 SP DMA, emitted late enough that
every store's DMA request reaches the device FIFO after the last load
request (the FIFO is grant-ordered; an early store would push loads
back). HWDGE-lane issue order: loads first, stores after.
"""

import sys

_TRN_REPO = "/opt/trn_rl_repo"
if _TRN_REPO not in sys.path:
    sys.path.insert(0, _TRN_REPO)

import numpy as np

import concourse.bass as bass
import concourse.mybir as mybir
from concourse.tile import TileContext

B, C, H, W = 16, 768, 64, 48
HO, WO = 7, 7
NCORES = 8
ROWS = B * C // NCORES  # 1536
P = 128
NTILES = ROWS // P  # 12
WD = 39  # steady: DVE H cols [0, WD), Pool cols [WD, 48)
f32 = mybir.dt.float32
X = mybir.AxisListType.X
XY = mybir.AxisListType.XY

_nc_cache = None


def _legalize_multiwait(nc: bass.Bass) -> None:
    """Walrus accepts at most one sync wait per instruction (two for
    EventSemaphore). Hoist extra waits into single-wait EventSemaphore
    carriers placed directly before the offending instruction."""
    n = 0
    final_lane = None
    for b in nc.m.functions[0].blocks:
        for inst in b.instructions:
            if type(inst).__name__ == "InstDMACopy" and inst.sync_info:
                for w in inst.sync_info.on_wait:
                    if (w.ant_name or "").startswith("DMAHW"):
                        final_lane = w.ant_name
    for b in nc.m.functions[0].blocks:
        insts = b.instructions
        i = 0
        while i < len(insts):
            inst = insts[i]
            si = inst.sync_info
            if si is not None and len(si.on_wait) > 1:
                waits = sorted(
                    si.on_wait,
                    key=lambda w: (
                        2
                        if w.ant_name == final_lane
                        else 1
                        if (w.ant_name or "").startswith("DMA")
                        else 0
                    ),
                )
                carriers = []
                rest = waits[:-1]
                for j in range(0, len(rest), 2):
                    n += 1
                    ev = mybir.InstEventSemaphore(
                        name=f"I-waitfix-{n}", ins=[], outs=[]
                    )
                    ev.engine = inst.engine
                    ev.sync_info = mybir.SyncInfo(
                        on_wait=rest[j : j + 2], on_update=[]
                    )
                    nc.register_instruction(ev)
                    carriers.append(ev)
                inst.sync_info = mybir.SyncInfo(
                    on_wait=[waits[-1]], on_update=list(si.on_update)
                )
                insts[i:i] = carriers
                i += len(carriers)
            i += 1



def _strip_pool_ring_memsets(nc: bass.Bass) -> None:
    """The framework preamble memsets the SWDGE descriptor rings on Pool.
    This kernel issues no SWDGE DMAs (loads ride SP, stores Act/SP via
    HWDGE), so the ring init only delays the entry barrier; drop it."""
    for b in nc.m.functions[0].blocks:
        keep = [
            i
            for i in b.instructions
            if not (
                type(i).__name__ == "InstMemset"
                and str(i.engine).endswith("Pool")
                and i.name in ("I-29", "I-30", "I-31", "I-32")
            )
        ]
        if len(keep) != len(b.instructions):
            b.instructions[:] = keep



def _hoist_first_load(nc: bass.Bass) -> None:
    """Move the first load's dma_start into the preamble block, right after
    SP's queue-setup RegisterMoves and before the entry barrier. The load
    has no waits and its consumers are semaphore-gated, so it can issue
    while the other engines are still running their preambles (~0.5us
    earlier DMA start)."""
    blocks = nc.m.functions[0].blocks
    if len(blocks) < 2:
        return
    b0, b1 = blocks[0], blocks[1]
    first = None
    for i in b1.instructions:
        if type(i).__name__ == "InstDMACopy" and str(i.engine).endswith("SP"):
            if i.sync_info and i.sync_info.on_wait:
                return  # unexpected: keep conservative
            first = i
            break
    if first is None:
        return
    b1.instructions.remove(first)
    idx = next(
        (k for k, i in enumerate(b0.instructions) if type(i).__name__ == "InstDrain"),
        None,
    )
    if idx is None:
        b1.instructions.insert(0, first)
        return
    b0.instructions.insert(idx, first)


def _build() -> bass.Bass:
    nc = bass.Bass()
    x = nc.dram_tensor("x", [ROWS, H * W], f32, kind="ExternalInput")
    out = nc.dram_tensor("out", [ROWS, HO * WO], f32, kind="ExternalOutput")
    ACopy = mybir.ActivationFunctionType.Copy
    with TileContext(nc) as tc:
        with (
            tc.tile_pool(name="xp", bufs=NTILES - 2) as xp,
            tc.tile_pool(name="cp", bufs=1) as cpool,
            tc.tile_pool(name="tp", bufs=3) as tp,
            tc.tile_pool(name="wp", bufs=3) as wp,
            tc.tile_pool(name="op", bufs=1) as op,
        ):
            nfull = NTILES - 2
            os_ = op.tile([P, NTILES * HO * WO], f32)  # [128, 588] staging
            po = list(os_.ap[0])

            # --- loads: 10 full tiles; tile 10 as o0-5 + o6 h-chunks;
            #     tile 11 as o0-2 / o3-5 / o6 h-chunks ---------------------
            xt = []
            for i in range(nfull):
                t = xp.tile([P, H, W], f32)
                nc.sync.dma_start(
                    out=t,
                    in_=x[i * P : (i + 1) * P, :].rearrange(
                        "p (h w) -> p h w", w=W
                    ),
                )
                xt.append(t)
            rows10 = x[nfull * P : (nfull + 1) * P, :].rearrange(
                "p (h w) -> p h w", w=W
            )
            xa10 = cpool.tile([P, 55, W], f32, tag="xa10")
            nc.sync.dma_start(out=xa10, in_=rows10[:, 0:55, :])
            xc10 = cpool.tile([P, 10, W], f32, tag="xc10")
            nc.sync.dma_start(out=xc10, in_=rows10[:, 54:64, :])
            rows11 = x[(nfull + 1) * P :, :].rearrange("p (h w) -> p h w", w=W)
            xa = cpool.tile([P, 28, W], f32, tag="xa")
            nc.sync.dma_start(out=xa, in_=rows11[:, 0:28, :])
            xb = cpool.tile([P, 28, W], f32, tag="xb")
            nc.sync.dma_start(out=xb, in_=rows11[:, 27:55, :])
            xc = cpool.tile([P, 10, W], f32, tag="xc")
            nc.sync.dma_start(out=xc, in_=rows11[:, 54:64, :])

            # --- helpers -------------------------------------------------
            def h_pool(xtile, tH):
                pt = list(xtile.ap[0])
                ph = list(tH.ap[0])
                nc.vector.reduce_sum(
                    out=bass.AP(
                        tensor=tH.tensor,
                        offset=tH.offset,
                        ap=[ph, [W, HO], [1, WD]],
                    ),
                    in_=bass.AP(
                        tensor=xtile.tensor,
                        offset=xtile.offset,
                        ap=[pt, [9 * W, HO], [1, WD], [W, 10]],
                    ),
                    axis=X,
                )
                tsl = bass.AP(
                    tensor=tH.tensor,
                    offset=tH.offset + WD,
                    ap=[ph, [W, HO], [1, W - WD]],
                )

                def xsl(k):
                    return bass.AP(
                        tensor=xtile.tensor,
                        offset=xtile.offset + k * W + WD,
                        ap=[pt, [9 * W, HO], [1, W - WD]],
                    )

                nc.gpsimd.tensor_add(tsl, xsl(0), xsl(1))
                for k in range(2, 10):
                    nc.gpsimd.tensor_add(tsl, tsl, xsl(k))

            def w_pool(tH, wout):
                ph = list(tH.ap[0])
                pw = list(wout.ap[0])
                nc.vector.reduce_sum(
                    out=bass.AP(
                        tensor=wout.tensor,
                        offset=wout.offset,
                        ap=[pw, [WO, HO], [6, 2]],
                    ),
                    in_=bass.AP(
                        tensor=tH.tensor,
                        offset=tH.offset,
                        ap=[ph, [W, HO], [41, 2], [1, 7]],
                    ),
                    axis=X,
                )
                nc.vector.reduce_sum(
                    out=bass.AP(
                        tensor=wout.tensor,
                        offset=wout.offset + 1,
                        ap=[pw, [WO, HO], [1, 5]],
                    ),
                    in_=bass.AP(
                        tensor=tH.tensor,
                        offset=tH.offset + 6,
                        ap=[ph, [W, HO], [7, 5], [1, 8]],
                    ),
                    axis=X,
                )

            def act_scales(wout, obase, o0, no):
                pw = list(wout.ap[0])
                nc.scalar.activation(
                    out=bass.AP(
                        tensor=os_.tensor,
                        offset=os_.offset + obase + 7 * o0,
                        ap=[po, [WO, no], [6, 2]],
                    ),
                    in_=bass.AP(
                        tensor=wout.tensor,
                        offset=wout.offset + 7 * o0,
                        ap=[pw, [WO, no], [6, 2]],
                    ),
                    func=ACopy,
                    scale=1.0 / 70.0,
                )
                nc.scalar.activation(
                    out=bass.AP(
                        tensor=os_.tensor,
                        offset=os_.offset + obase + 7 * o0 + 1,
                        ap=[po, [WO, no], [1, 5]],
                    ),
                    in_=bass.AP(
                        tensor=wout.tensor,
                        offset=wout.offset + 7 * o0 + 1,
                        ap=[pw, [WO, no], [1, 5]],
                    ),
                    func=ACopy,
                    scale=1.0 / 80.0,
                )

            def chunk_hw(xtile, tH, wout, o0, no, h0, do_w=True):
                """H-pool rows [o0,o0+no) split DVE/Pool, then W-pool."""
                pt = list(xtile.ap[0])
                ph = list(tH.ap[0])
                pw = list(wout.ap[0])
                off = (9 * o0 - h0) * W
                nc.vector.reduce_sum(
                    out=bass.AP(
                        tensor=tH.tensor,
                        offset=tH.offset + o0 * W,
                        ap=[ph, [W, no], [1, WD]],
                    ),
                    in_=bass.AP(
                        tensor=xtile.tensor,
                        offset=xtile.offset + off,
                        ap=[pt, [9 * W, no], [1, WD], [W, 10]],
                    ),
                    axis=X,
                )
                tsl = bass.AP(
                    tensor=tH.tensor,
                    offset=tH.offset + o0 * W + WD,
                    ap=[ph, [W, no], [1, W - WD]],
                )

                def xsl(k):
                    return bass.AP(
                        tensor=xtile.tensor,
                        offset=xtile.offset + off + k * W + WD,
                        ap=[pt, [9 * W, no], [1, W - WD]],
                    )

                nc.gpsimd.tensor_add(tsl, xsl(0), xsl(1))
                for k in range(2, 10):
                    nc.gpsimd.tensor_add(tsl, tsl, xsl(k))
                if not do_w:
                    return
                nc.vector.reduce_sum(
                    out=bass.AP(
                        tensor=wout.tensor,
                        offset=wout.offset + 7 * o0,
                        ap=[pw, [WO, no], [6, 2]],
                    ),
                    in_=bass.AP(
                        tensor=tH.tensor,
                        offset=tH.offset + o0 * W,
                        ap=[ph, [W, no], [41, 2], [1, 7]],
                    ),
                    axis=X,
                )
                nc.vector.reduce_sum(
                    out=bass.AP(
                        tensor=wout.tensor,
                        offset=wout.offset + 7 * o0 + 1,
                        ap=[pw, [WO, no], [1, 5]],
                    ),
                    in_=bass.AP(
                        tensor=tH.tensor,
                        offset=tH.offset + o0 * W + 6,
                        ap=[ph, [W, no], [7, 5], [1, 8]],
                    ),
                    axis=X,
                )

            def fused_chunk(xtile, wout, o0, no, h0):
                """wout[:, 7o+q] for o in [o0,o0+no) by 2D (h,w) reduces."""
                pc = list(xtile.ap[0])
                pw = list(wout.ap[0])
                off = (9 * o0 - h0) * W
                nc.vector.reduce_sum(
                    out=bass.AP(
                        tensor=wout.tensor,
                        offset=wout.offset + 7 * o0,
                        ap=[pw, [WO, no], [6, 2]],
                    ),
                    in_=bass.AP(
                        tensor=xtile.tensor,
                        offset=xtile.offset + off,
                        ap=[pc, [9 * W, no], [41, 2], [W, 10], [1, 7]],
                    ),
                    axis=XY,
                )
                nc.vector.reduce_sum(
                    out=bass.AP(
                        tensor=wout.tensor,
                        offset=wout.offset + 7 * o0 + 1,
                        ap=[pw, [WO, no], [1, 5]],
                    ),
                    in_=bass.AP(
                        tensor=xtile.tensor,
                        offset=xtile.offset + off + 6,
                        ap=[pc, [9 * W, no], [7, 5], [W, 10], [1, 8]],
                    ),
                    axis=XY,
                )

            # --- steady tiles 0..10 -------------------------------------
            for i in range(nfull):
                tH = tp.tile([P, HO, W], f32)
                wout = wp.tile([P, HO * WO], f32)
                h_pool(xt[i], tH)
                w_pool(tH, wout)
                act_scales(wout, i * HO * WO, 0, HO)
                if i == 8:
                    # Batched store, tiles 0..7. Emitted here so its DMA
                    # request trails every load request; its data (scales
                    # 0..7) is already ordered on Act.
                    nc.scalar.dma_start(
                        out=bass.AP(
                            tensor=out,
                            offset=0,
                            ap=[[HO * WO, P], [HO * WO * P, 8], [1, HO * WO]],
                        ),
                        in_=bass.AP(
                            tensor=os_.tensor,
                            offset=os_.offset,
                            ap=[po, [HO * WO, 8], [1, HO * WO]],
                        ),
                    )

            # --- tile 10 as two chunks ----------------------------------
            wout10 = wp.tile([P, HO * WO], f32)
            tH10 = tp.tile([P, HO, W], f32)
            obase10 = nfull * HO * WO
            chunk_hw(xa10, tH10, wout10, 0, 6, 0)
            act_scales(wout10, obase10, 0, 6)
            fused_chunk(xc10, wout10, 6, 1, 54)
            act_scales(wout10, obase10, 6, 1)
            # --- tile 11 chunks -----------------------------------------
            wout11 = wp.tile([P, HO * WO], f32)
            tH11 = tp.tile([P, HO, W], f32)
            obase11 = (nfull + 1) * HO * WO
            chunk_hw(xa, tH11, wout11, 0, 3, 0, do_w=False)
            chunk_hw(xb, tH11, wout11, 3, 3, 27, do_w=False)
            pw11 = list(wout11.ap[0])
            ph11 = list(tH11.ap[0])
            nc.vector.reduce_sum(
                out=bass.AP(
                    tensor=wout11.tensor,
                    offset=wout11.offset,
                    ap=[pw11, [WO, 6], [6, 2]],
                ),
                in_=bass.AP(
                    tensor=tH11.tensor,
                    offset=tH11.offset,
                    ap=[ph11, [W, 6], [41, 2], [1, 7]],
                ),
                axis=X,
            )
            nc.vector.reduce_sum(
                out=bass.AP(
                    tensor=wout11.tensor,
                    offset=wout11.offset + 1,
                    ap=[pw11, [WO, 6], [1, 5]],
                ),
                in_=bass.AP(
                    tensor=tH11.tensor,
                    offset=tH11.offset + 6,
                    ap=[ph11, [W, 6], [7, 5], [1, 8]],
                ),
                axis=X,
            )
            act_scales(wout11, obase11, 0, 6)
            # tiles 8..10 full-block store once tile 10's scales are done
            nc.scalar.dma_start(
                out=bass.AP(
                    tensor=out,
                    offset=8 * HO * WO * P,
                    ap=[[HO * WO, P], [HO * WO * P, 3], [1, HO * WO]],
                ),
                in_=bass.AP(
                    tensor=os_.tensor,
                    offset=os_.offset + 8 * HO * WO,
                    ap=[po, [HO * WO, 3], [1, HO * WO]],
                ),
            )
            # o6: fused reduce + DVE scales + SP store (the tail chain)
            pc = list(xc.ap[0])
            pw = list(wout11.ap[0])
            nc.vector.reduce_sum(
                out=bass.AP(
                    tensor=wout11.tensor,
                    offset=wout11.offset + 42,
                    ap=[pw, [6, 2]],
                ),
                in_=bass.AP(
                    tensor=xc.tensor,
                    offset=xc.offset,
                    ap=[pc, [41, 2], [W, 10], [1, 7]],
                ),
                axis=XY,
            )
            nc.vector.reduce_sum(
                out=bass.AP(
                    tensor=wout11.tensor,
                    offset=wout11.offset + 43,
                    ap=[pw, [1, 5]],
                ),
                in_=bass.AP(
                    tensor=xc.tensor,
                    offset=xc.offset + 6,
                    ap=[pc, [7, 5], [W, 10], [1, 8]],
                ),
                axis=XY,
            )
            nc.vector.tensor_scalar_mul(
                bass.AP(
                    tensor=os_.tensor,
                    offset=os_.offset + obase11 + 42,
                    ap=[po, [6, 2]],
                ),
                bass.AP(
                    tensor=wout11.tensor,
                    offset=wout11.offset + 42,
                    ap=[pw, [6, 2]],
                ),
                1.0 / 70.0,
            )
            nc.vector.tensor_scalar_mul(
                bass.AP(
                    tensor=os_.tensor,
                    offset=os_.offset + obase11 + 43,
                    ap=[po, [1, 5]],
                ),
                bass.AP(
                    tensor=wout11.tensor,
                    offset=wout11.offset + 43,
                    ap=[pw, [1, 5]],
                ),
                1.0 / 80.0,
            )
            nc.sync.dma_start(
                out=out[(nfull + 1) * P :, :],
                in_=os_[:, obase11 : obase11 + HO * WO],
            )
    _strip_pool_ring_memsets(nc)
    _legalize_multiwait(nc)
    _hoist_first_load(nc)
    return nc


def kernel(x: np.ndarray) -> np.ndarray:
    global _nc_cache
    from concourse.bass_utils import run_bass_kernel_spmd

    xr = np.ascontiguousarray(
        np.asarray(x, dtype=np.float32).reshape(B * C, H * W)
    )
    if _nc_cache is None:
        _nc_cache = _build()
    nc = _nc_cache
    in_maps = [{"x": xr[k * ROWS : (k + 1) * ROWS]} for k in range(NCORES)]
    res = run_bass_kernel_spmd(nc, in_maps, list(range(NCORES)))
    out = np.concatenate([r["out"] for r in res.results], axis=0)
    return out.reshape(B, C, HO, WO)
